# revision 7
# baseline (speedup 1.0000x reference)
"""Trainium2 Bass kernel for nn_DPS_topk (topk_masking).

Forward pass equals `hard`: one-hot expansion along D of the top-16
indices of (logits + gn), k axis ordered by ascending index.

v4 pipeline (per 128-row tile, rows on partitions):
    x    = logits + gn                       (f32, exact)
    v8   = max8(x); i8a = max_index(v8, x)   (top-8 values + positions)
    x2   = match_replace8(v8, x, -1e30)
    v16  = max8(x2); i8b = max_index(v16, x2)  (ranks 9..16)
    ineg = -(i8a | i8b)                      (fp16; ints <= 1023 exact)
    h8   = max8(ineg); l8 = max8(match_replace8(h8, ineg, -inf))
         -> h8[:,j] = -(j-th smallest index), l8[:,j] = -(8+j-th smallest)
    plane j = is_eq(-iota, h8/l8[:, j])      (DVE 4x, fp16 in, ~335 ns)

This removes v3's is_ge + tensor_tensor_scan + mult (3.7 us/tile; the
scan alone is 2.3 us and has no fast mode).

Output: one-hot values are exactly {0,1}, so the DRAM output is
narrow and the host widens losslessly to f32.  Tile 0 and tile 1
planes 0-5 are fp16 (DVE 4x production feeds the write stream early);
tile 1 planes 6-15 are uint8 into a second DRAM tensor -- by then the
kernel is draining the final write stream, so halving those bytes
(u8 planes cost 594 ns on DVE but the DVE is otherwise idle at the
end) shortens the exposed tail.

Other structure (from v1-v3 traces):
  * inputs ride three DMA rings: gn tile0 (column-split) on sync HWDGE,
    logits p0-63 on scalar HWDGE, logits p64-127 on gpsimd SWDGE.
  * tile-1's add is a SWDGE accumulate-DMA (gn tile1 += logits buffer)
    issued after the tile-0 add consumes the buffer: no DVE cost.
  * gpsimd also precomputes -iota (u16 iota, negate to fp16) before
    inputs land.
  * output plane-group DMAs gate on the plane-completion semaphore
    with 2 planes of slack (a plane's sem-inc fires ~0.35 us before
    its posted writes drain); each tile's final group gates on the
    pipe-empty drain marker.  Small leading groups start each stream
    ~1 us after the ranks exist.

Raw Bass: one sync-wait per instruction; explicit drains between
dependent same-engine ops (hardware-verified requirement).
"""

import numpy as np

K = 16
D = 1024
N = 64
BS = 32
NCORES = 8
BS_PER_CORE = BS // NCORES   # 4
ROWS = BS_PER_CORE * N       # 256 rows per core
P = 128                      # SBUF partitions
NTILES = ROWS // P           # 2

NF1 = 6                      # tile-1 planes 0..NF1-1 are fp16, rest u8
PER_TILE = K + 1             # 16 plane incs + 1 drain inc on cmp_sem
SLACK = 2

# (start_plane, n_planes) groups; tile0 all fp16
GROUPS0 = [(0, 1), (1, 1), (2, 2), (4, 4), (8, 4), (12, 2), (14, 1), (15, 1)]
GROUPS1F = [(0, 1), (1, 1), (2, 2), (4, 2)]            # tile1 fp16 part
GROUPS1B = [(6, 4), (10, 4), (14, 2)]                  # tile1 u8 part

_CACHE = {}


def _gate(i, s, l):
    return PER_TILE * i + min(s + l + SLACK, PER_TILE)


def _build_nc():
    import concourse.bass as bass
    from concourse import mybir

    f32 = mybir.dt.float32
    f16 = mybir.dt.float16
    u16 = mybir.dt.uint16
    u8 = mybir.dt.uint8
    A = mybir.AluOpType

    nc = bass.Bass()
    lg_d = nc.declare_dram_parameter("logits", [N, D], f32, isOutput=False)
    gn_d = nc.declare_dram_parameter("gn", [ROWS, D], f32, isOutput=False)
    # out_a: tile0 all planes + tile1 planes 0..NF1-1 (fp16)
    out_a = nc.declare_dram_parameter("out_a", [ROWS, K * D], f16, isOutput=True)
    # out_b: tile1 planes NF1..15 (u8)
    out_b = nc.declare_dram_parameter(
        "out_b", [P, (K - NF1) * D], u8, isOutput=True
    )

    from contextlib import ExitStack

    with ExitStack() as stack:
        e = stack.enter_context
        gt0 = e(nc.sbuf_tensor([P, D], f32))
        lg = e(nc.sbuf_tensor([P, D], f32))   # logits; later x1 = lg + gn1
        x = e(nc.sbuf_tensor([P, D], f32))
        x2 = e(nc.sbuf_tensor([P, D], f32))
        v8 = e(nc.sbuf_tensor([P, 8], f32))
        v16 = e(nc.sbuf_tensor([P, 8], f32))
        idx = e(nc.sbuf_tensor([P, 16], u16))
        ineg = e(nc.sbuf_tensor([P, 16], f32))
        ineg2 = e(nc.sbuf_tensor([P, 16], f32))
        h8 = e(nc.sbuf_tensor([P, 8], f32))
        l8 = e(nc.sbuf_tensor([P, 8], f32))
        iota_u = e(nc.sbuf_tensor([P, D], u16))
        iota_n = e(nc.sbuf_tensor([P, D], f16))
        chunk0 = e(nc.sbuf_tensor([P, K * D], f16))
        chunk1f = e(nc.sbuf_tensor([P, NF1 * D], f16))
        chunk1b = e(nc.sbuf_tensor([P, (K - NF1) * D], u8))
        in0a_sem = e(nc.semaphore("in0a_sem"))   # gn0 left half + logits p0-63
        lg2_sem = e(nc.semaphore("lg2_sem"))     # logits p64-127 (SWDGE)
        in0b_sem = e(nc.semaphore("in0b_sem"))   # gn0 right half
        iota_sem = e(nc.semaphore("iota_sem"))   # -iota ready (gpsimd)
        xfree_sem = e(nc.semaphore("xfree_sem")) # tile-0 add consumed lg
        in1_sem = e(nc.semaphore("in1_sem"))     # SWDGE accum gn1 done
        cmp_sem = e(nc.semaphore("cmp_sem"))
        dma_sem = e(nc.semaphore("dma_sem"))
        block = e(nc.Block(no_gpsimd_drain=True))
        H = D // 2
        NDMA = len(GROUPS0) + len(GROUPS1F) + len(GROUPS1B)

        @block.scalar
        def _(scalar: "bass.BassEngine"):
            scalar.dma_start(out=lg[0:N, :], in_=lg_d[:, :]).then_inc(in0a_sem, 16)
            # tile-1 output groups
            for s, l in GROUPS1F:
                scalar.wait_ge(cmp_sem, _gate(1, s, l))
                scalar.dma_start(
                    out=out_a[P : 2 * P, D * s : D * (s + l)],
                    in_=chunk1f[:, D * s : D * (s + l)],
                ).then_inc(dma_sem, 16)
            for s, l in GROUPS1B[:-1]:
                scalar.wait_ge(cmp_sem, _gate(1, s, l))
                scalar.dma_start(
                    out=out_b[:, D * (s - NF1) : D * (s - NF1 + l)],
                    in_=chunk1b[:, D * (s - NF1) : D * (s - NF1 + l)],
                ).then_inc(dma_sem, 16)

        @block.gpsimd
        def _(gpsimd: "bass.BassEngine"):
            # logits p64-127 via SWDGE first -- it gates the tile-0 add;
            # the iota is only needed when planes start (~8 us later)
            gpsimd.dma_start(out=lg[N:P, :], in_=lg_d[:, :]).then_inc(lg2_sem, 16)
            # precompute -iota (free: runs while inputs land)
            gpsimd.iota(iota_u[:], pattern=[[1, D]], base=0, channel_multiplier=0)
            gpsimd.drain()
            gpsimd.tensor_scalar(
                iota_n[:], iota_u[:], -1.0, None, op0=A.mult
            ).then_inc(iota_sem, 1)
            # tile-1 add for free: gn rows 128..255 accumulate onto lg
            gpsimd.wait_ge(xfree_sem, 1)
            gpsimd.dma_start(
                out=lg[:, :], in_=gn_d[P : 2 * P, :], accum_op=A.add
            ).then_inc(in1_sem, 16)

        @block.sync
        def _(sync: "bass.BassEngine"):
            sync.dma_start(out=gt0[:, 0:H], in_=gn_d[0:P, 0:H]).then_inc(
                in0a_sem, 16
            )
            sync.dma_start(out=gt0[:, H:D], in_=gn_d[0:P, H:D]).then_inc(
                in0b_sem, 16
            )
            for s, l in GROUPS0:
                sync.wait_ge(cmp_sem, _gate(0, s, l))
                sync.dma_start(
                    out=out_a[0:P, D * s : D * (s + l)],
                    in_=chunk0[:, D * s : D * (s + l)],
                ).then_inc(dma_sem, 16)
            s, l = GROUPS1B[-1]
            sync.wait_ge(cmp_sem, _gate(1, s, l))
            sync.dma_start(
                out=out_b[:, D * (s - NF1) : D * (s - NF1 + l)],
                in_=chunk1b[:, D * (s - NF1) : D * (s - NF1 + l)],
            ).then_inc(dma_sem, 16)
            sync.wait_ge(dma_sem, 16 * NDMA)

        @block.vector
        def _(vector: "bass.BassEngine"):
            def dr():
                vector.drain()

            xs = [x, lg]
            for i in range(NTILES):
                if i == 0:
                    vector.wait_ge(in0a_sem, 32)
                    vector.wait_ge(lg2_sem, 16)
                    vector.tensor_tensor(
                        x[:, 0:H], gt0[:, 0:H], lg[:, 0:H], op=A.add
                    )
                    vector.wait_ge(in0b_sem, 16)
                    vector.tensor_tensor(
                        x[:, H:D], gt0[:, H:D], lg[:, H:D], op=A.add
                    ).then_inc(xfree_sem, 1)
                    dr()
                else:
                    vector.wait_ge(in1_sem, 16)
                xi = xs[i][:]
                vector.max(v8[:], xi)
                dr()
                vector.max_index(idx[:, 0:8], v8[:], xi)
                dr()
                vector.match_replace(x2[:], v8[:], xi, -1e30)
                dr()
                vector.max(v16[:], x2[:])
                dr()
                vector.max_index(idx[:, 8:16], v16[:], x2[:])
                dr()
                # negate the 16 selected positions into fp16 (exact)
                vector.tensor_scalar(ineg[:], idx[:], -1.0, None, op0=A.mult)
                dr()
                # sort: h8 = 8 largest of -idx = -(8 smallest indices) desc
                vector.max(h8[:], ineg[:])
                dr()
                vector.match_replace(ineg2[:], h8[:], ineg[:], -1e30)
                dr()
                vector.max(l8[:], ineg2[:])
                dr()
                if i == 0:
                    vector.wait_ge(iota_sem, 1)
                # plane j one-hot: -iota == -(j-th smallest index)
                for j in range(K):
                    src = h8[:, j : j + 1] if j < 8 else l8[:, j - 8 : j - 7]
                    if i == 0:
                        dst = chunk0[:, D * j : D * (j + 1)]
                    elif j < NF1:
                        dst = chunk1f[:, D * j : D * (j + 1)]
                    else:
                        dst = chunk1b[:, D * (j - NF1) : D * (j - NF1 + 1)]
                    vector.tensor_scalar(
                        dst, iota_n[:], src, None, op0=A.is_equal
                    ).then_inc(cmp_sem, 1)
                vector.drain().then_inc(cmp_sem, 1)

    return nc


def _get_nc():
    if "nc" not in _CACHE:
        _CACHE["nc"] = _build_nc()
    return _CACHE["nc"]


def kernel(logits: np.ndarray, gn: np.ndarray) -> np.ndarray:
    from concourse.bass_utils import run_bass_kernel_spmd

    logits = np.ascontiguousarray(np.asarray(logits, dtype=np.float32))
    gn = np.asarray(gn, dtype=np.float32)
    assert logits.shape == (N, D) and gn.shape == (BS, N, D)

    nc = _get_nc()
    in_maps = []
    for c in range(NCORES):
        shard = np.ascontiguousarray(
            gn[c * BS_PER_CORE : (c + 1) * BS_PER_CORE].reshape(ROWS, D)
        )
        in_maps.append({"logits": logits, "gn": shard})

    res = run_bass_kernel_spmd(nc, in_maps, list(range(NCORES))).results
    outs = []
    for r in res:
        full = r["out_a"].astype(np.float32)          # [256, K*D]
        full[P:, NF1 * D :] = r["out_b"].astype(np.float32)
        outs.append(full.reshape(BS_PER_CORE, N, K, D))
    return np.concatenate(outs, axis=0)


# revision 8
# speedup vs baseline: 1.5459x; 1.5459x over previous
"""Trainium2 Bass kernel for nn_DPS_topk (topk_masking).

Forward pass equals `hard`: one-hot expansion along D of the top-16
indices of (logits + gn), k axis ordered by ascending index.

Per row x (length D=1024), all on DVE (v4 showed max_index+PTR-scalar
planes lose their 4x mode; immediate-constant planes are faster):
    x   = logits + gn                        (f32, exact)
    t   = 16th largest via max8 / match_replace8 / max8
    m   = (x >= t)                           (fp16 mask)
    q   = inclusive_cumsum(m) * m            (fp16; scan accumulates f32)
    plane j = (q == j+1)                     (imm constant, DVE 4x ~335 ns)

Output: values are exactly {0,1}; DRAM output is narrow and the host
widens losslessly to f32.  Tile 0 and tile 1 planes 0-7 are fp16; tile
1 planes 8-15 are uint8 into a second DRAM tensor: the final stream is
the exposed tail of the kernel (v3 spent 8.5 us draining it), and at
that point the DVE is nearly done, so trading slower u8 plane ops
(686 ns, 2x mode) for half the bytes shortens the critical path.

Structure (from v1-v4 traces):
  * inputs ride three DMA rings in parallel: gn tile0 (column-split so
    the add starts on the first half) on sync HWDGE, logits p0-63 on
    scalar HWDGE, logits p64-127 on gpsimd SWDGE (issued first -- v4
    showed gpsimd compute before it delays everything).
  * tile-1's add is a SWDGE accumulate-DMA (gn tile1 += logits buffer)
    issued after the tile-0 add consumes the buffer: no DVE cost.
  * output plane-group DMAs gate on the plane-completion semaphore with
    2 planes of slack (a plane's sem-inc fires ~0.35 us before its
    posted writes drain); final groups gate on the pipe-empty drain
    marker.  Small leading groups start each tile's stream ~1 us after
    q exists; the two final groups sit on different rings.

Raw Bass: one sync-wait per instruction; explicit drains between
dependent same-engine DVE ops (hardware-verified requirement).
"""

import numpy as np

K = 16
D = 1024
N = 64
BS = 32
NCORES = 8
BS_PER_CORE = BS // NCORES   # 4
ROWS = BS_PER_CORE * N       # 256 rows per core
P = 128                      # SBUF partitions
NTILES = ROWS // P           # 2

NF1 = 8                      # tile-1 planes 0..NF1-1 fp16, rest u8
PER_TILE = K + 1             # 16 plane incs + 1 drain inc on cmp_sem
SLACK = 2

GROUPS0 = [(0, 1), (1, 1), (2, 2), (4, 4), (8, 4), (12, 2), (14, 1), (15, 1)]
GROUPS1F = [(0, 1), (1, 1), (2, 2), (4, 4)]   # tile1 fp16 part -> out_a
GROUPS1B = [(8, 4), (12, 3), (15, 1)]         # tile1 u8 part -> out_b

_CACHE = {}


def _gate(i, s, l):
    return PER_TILE * i + min(s + l + SLACK, PER_TILE)


def _build_nc():
    import concourse.bass as bass
    from concourse import mybir
    from contextlib import ExitStack

    f32 = mybir.dt.float32
    f16 = mybir.dt.float16
    u8 = mybir.dt.uint8
    A = mybir.AluOpType

    nc = bass.Bass()
    lg_d = nc.declare_dram_parameter("logits", [N, D], f32, isOutput=False)
    gn_d = nc.declare_dram_parameter("gn", [ROWS, D], f32, isOutput=False)
    out_a = nc.declare_dram_parameter("out_a", [ROWS, K * D], f16, isOutput=True)
    out_b = nc.declare_dram_parameter(
        "out_b", [P, (K - NF1) * D], u8, isOutput=True
    )

    with ExitStack() as stack:
        e = stack.enter_context
        gt0 = e(nc.sbuf_tensor([P, D], f32))
        lg = e(nc.sbuf_tensor([P, D], f32))   # logits; later x1 = lg + gn1
        x = e(nc.sbuf_tensor([P, D], f32))
        x2 = e(nc.sbuf_tensor([P, D], f32))
        v8 = e(nc.sbuf_tensor([P, 8], f32))
        v16 = e(nc.sbuf_tensor([P, 8], f32))
        m = e(nc.sbuf_tensor([P, D], f16))
        qi = e(nc.sbuf_tensor([P, D], f16))
        q0 = e(nc.sbuf_tensor([P, D], f16))
        q1 = e(nc.sbuf_tensor([P, D], f16))
        chunk0 = e(nc.sbuf_tensor([P, K * D], f16))
        chunk1f = e(nc.sbuf_tensor([P, NF1 * D], f16))
        chunk1b = e(nc.sbuf_tensor([P, (K - NF1) * D], u8))
        in0a_sem = e(nc.semaphore("in0a_sem"))   # gn0 left + logits p0-63
        lg2_sem = e(nc.semaphore("lg2_sem"))     # logits p64-127 (SWDGE)
        in0b_sem = e(nc.semaphore("in0b_sem"))   # gn0 right half
        xfree_sem = e(nc.semaphore("xfree_sem")) # tile-0 add consumed lg
        in1_sem = e(nc.semaphore("in1_sem"))     # SWDGE accum gn1 done
        cmp_sem = e(nc.semaphore("cmp_sem"))
        dma_sem = e(nc.semaphore("dma_sem"))
        block = e(nc.Block(no_gpsimd_drain=True))

        H = D // 2
        NDMA = len(GROUPS0) + len(GROUPS1F) + len(GROUPS1B)

        @block.scalar
        def _(scalar: "bass.BassEngine"):
            scalar.dma_start(out=lg[0:N, :], in_=lg_d[:, :]).then_inc(in0a_sem, 16)
            for s, l in GROUPS1F:
                scalar.wait_ge(cmp_sem, _gate(1, s, l))
                scalar.dma_start(
                    out=out_a[P : 2 * P, D * s : D * (s + l)],
                    in_=chunk1f[:, D * s : D * (s + l)],
                ).then_inc(dma_sem, 16)
            for s, l in GROUPS1B[:-1]:
                scalar.wait_ge(cmp_sem, _gate(1, s, l))
                scalar.dma_start(
                    out=out_b[:, D * (s - NF1) : D * (s - NF1 + l)],
                    in_=chunk1b[:, D * (s - NF1) : D * (s - NF1 + l)],
                ).then_inc(dma_sem, 16)

        @block.gpsimd
        def _(gpsimd: "bass.BassEngine"):
            # logits p64-127 via SWDGE, parallel with the scalar-ring load
            gpsimd.dma_start(out=lg[N:P, :], in_=lg_d[:, :]).then_inc(lg2_sem, 16)
            # tile-1 add for free: gn rows 128..255 accumulate onto lg
            gpsimd.wait_ge(xfree_sem, 1)
            gpsimd.dma_start(
                out=lg[:, :], in_=gn_d[P : 2 * P, :], accum_op=A.add
            ).then_inc(in1_sem, 16)

        @block.sync
        def _(sync: "bass.BassEngine"):
            sync.dma_start(out=gt0[:, 0:H], in_=gn_d[0:P, 0:H]).then_inc(
                in0a_sem, 16
            )
            sync.dma_start(out=gt0[:, H:D], in_=gn_d[0:P, H:D]).then_inc(
                in0b_sem, 16
            )
            for s, l in GROUPS0:
                sync.wait_ge(cmp_sem, _gate(0, s, l))
                sync.dma_start(
                    out=out_a[0:P, D * s : D * (s + l)],
                    in_=chunk0[:, D * s : D * (s + l)],
                ).then_inc(dma_sem, 16)
            s, l = GROUPS1B[-1]
            sync.wait_ge(cmp_sem, _gate(1, s, l))
            sync.dma_start(
                out=out_b[:, D * (s - NF1) : D * (s - NF1 + l)],
                in_=chunk1b[:, D * (s - NF1) : D * (s - NF1 + l)],
            ).then_inc(dma_sem, 16)
            sync.wait_ge(dma_sem, 16 * NDMA)

        @block.vector
        def _(vector: "bass.BassEngine"):
            def dr():
                vector.drain()

            xs = [x, lg]
            qs = [q0, q1]
            for i in range(NTILES):
                if i == 0:
                    vector.wait_ge(in0a_sem, 32)
                    vector.wait_ge(lg2_sem, 16)
                    vector.tensor_tensor(
                        x[:, 0:H], gt0[:, 0:H], lg[:, 0:H], op=A.add
                    )
                    vector.wait_ge(in0b_sem, 16)
                    vector.tensor_tensor(
                        x[:, H:D], gt0[:, H:D], lg[:, H:D], op=A.add
                    ).then_inc(xfree_sem, 1)
                    dr()
                else:
                    vector.wait_ge(in1_sem, 16)
                xi = xs[i][:]
                vector.max(v8[:], xi)
                dr()
                vector.match_replace(x2[:], v8[:], xi, -1e30)
                dr()
                vector.max(v16[:], x2[:])
                dr()
                vector.tensor_scalar(m[:], xi, v16[:, 7:8], None, op0=A.is_ge)
                dr()
                vector.tensor_tensor_scan(
                    qi[:], m[:], m[:], 0.0, op0=A.add, op1=A.bypass
                )
                dr()
                vector.tensor_tensor(qs[i][:], qi[:], m[:], op=A.mult)
                dr()
                for j in range(K):
                    if i == 0:
                        dst = chunk0[:, D * j : D * (j + 1)]
                    elif j < NF1:
                        dst = chunk1f[:, D * j : D * (j + 1)]
                    else:
                        dst = chunk1b[:, D * (j - NF1) : D * (j - NF1 + 1)]
                    vector.tensor_scalar(
                        dst, qs[i][:], float(j + 1), None, op0=A.is_equal
                    ).then_inc(cmp_sem, 1)
                vector.drain().then_inc(cmp_sem, 1)

    return nc


def _get_nc():
    if "nc" not in _CACHE:
        _CACHE["nc"] = _build_nc()
    return _CACHE["nc"]


def kernel(logits: np.ndarray, gn: np.ndarray) -> np.ndarray:
    from concourse.bass_utils import run_bass_kernel_spmd

    logits = np.ascontiguousarray(np.asarray(logits, dtype=np.float32))
    gn = np.asarray(gn, dtype=np.float32)
    assert logits.shape == (N, D) and gn.shape == (BS, N, D)

    nc = _get_nc()
    in_maps = []
    for c in range(NCORES):
        shard = np.ascontiguousarray(
            gn[c * BS_PER_CORE : (c + 1) * BS_PER_CORE].reshape(ROWS, D)
        )
        in_maps.append({"logits": logits, "gn": shard})

    res = run_bass_kernel_spmd(nc, in_maps, list(range(NCORES))).results
    outs = []
    for r in res:
        full = r["out_a"].astype(np.float32)          # [256, K*D]
        full[P:, NF1 * D :] = r["out_b"].astype(np.float32)
        outs.append(full.reshape(BS_PER_CORE, N, K, D))
    return np.concatenate(outs, axis=0)


# revision 11
# speedup vs baseline: 1.6070x; 1.0396x over previous
"""Trainium2 Bass kernel for nn_DPS_topk (topk_masking).

Forward pass equals `hard`: one-hot expansion along D of the top-16
indices of (logits + gn), k axis ordered by ascending index.

Per row x (length D=1024), all on DVE (v4 showed max_index+PTR-scalar
planes lose their 4x mode; immediate-constant planes are faster):
    x   = logits + gn                        (f32, exact)
    t   = 16th largest via max8 / match_replace8 / max8
    m   = (x >= t)                           (fp16 mask)
    q   = inclusive_cumsum(m) * m            (fp16; scan accumulates f32)
    plane j = (q == j+1)                     (imm constant, DVE 4x ~335 ns)

Output: values are exactly {0,1}; DRAM output is narrow and the host
widens losslessly to f32.  Tile 0 and tile 1 planes 0-7 are fp16; tile
1 planes 8-15 are uint8 into a second DRAM tensor: the final stream is
the exposed tail of the kernel (v3 spent 8.5 us draining it), and at
that point the DVE is nearly done, so trading slower u8 plane ops
(686 ns, 2x mode) for half the bytes shortens the critical path.

Structure (from v1-v4 traces):
  * inputs ride three DMA rings in parallel: gn tile0 (column-split so
    the add starts on the first half) on sync HWDGE, logits p0-63 on
    scalar HWDGE, logits p64-127 on gpsimd SWDGE (issued first -- v4
    showed gpsimd compute before it delays everything).
  * tile-1's add is a SWDGE accumulate-DMA (gn tile1 += logits buffer)
    issued after the tile-0 add consumes the buffer: no DVE cost.
  * output plane-group DMAs gate on the plane-completion semaphore with
    2 planes of slack (a plane's sem-inc fires ~0.35 us before its
    posted writes drain); final groups gate on the pipe-empty drain
    marker.  Small leading groups start each tile's stream ~1 us after
    q exists; the two final groups sit on different rings.

Raw Bass: one sync-wait per instruction; explicit drains between
dependent same-engine DVE ops (hardware-verified requirement).
"""

import numpy as np

K = 16
D = 1024
N = 64
BS = 32
NCORES = 8
BS_PER_CORE = BS // NCORES   # 4
ROWS = BS_PER_CORE * N       # 256 rows per core
P = 128                      # SBUF partitions
NTILES = ROWS // P           # 2

NF1 = 8                      # tile-1 planes 0..NF1-1 fp16, rest u8
PER_TILE = K + 1             # 16 plane incs + 1 drain inc on cmp_sem
SLACK = 2

GROUPS0 = [(0, 1), (1, 1), (2, 2), (4, 4), (8, 4), (12, 2), (14, 1), (15, 1)]
GROUPS1F = [(0, 1), (1, 1), (2, 2), (4, 4)]   # tile1 fp16 part -> out_a
GROUPS1B = [(8, 4), (12, 3), (15, 1)]         # tile1 u8 part -> out_b

_CACHE = {}


def _gate(i, s, l):
    return PER_TILE * i + min(s + l + SLACK, PER_TILE)


def _build_nc():
    import concourse.bass as bass
    from concourse import mybir
    from contextlib import ExitStack

    f32 = mybir.dt.float32
    f16 = mybir.dt.float16
    u8 = mybir.dt.uint8
    A = mybir.AluOpType

    nc = bass.Bass()
    lg_d = nc.declare_dram_parameter("logits", [N, D], f32, isOutput=False)
    gn_d = nc.declare_dram_parameter("gn", [ROWS, D], f32, isOutput=False)
    out_a = nc.declare_dram_parameter("out_a", [ROWS, K * D], f16, isOutput=True)
    out_b = nc.declare_dram_parameter(
        "out_b", [P, (K - NF1) * D], u8, isOutput=True
    )

    with ExitStack() as stack:
        e = stack.enter_context
        gt0 = e(nc.sbuf_tensor([P, D], f32))
        lg = e(nc.sbuf_tensor([P, D], f32))   # logits; later x1 = lg + gn1
        x = e(nc.sbuf_tensor([P, D], f32))
        x2 = e(nc.sbuf_tensor([P, D], f32))
        v8 = e(nc.sbuf_tensor([P, 8], f32))
        v16 = e(nc.sbuf_tensor([P, 8], f32))
        m = e(nc.sbuf_tensor([P, D], f16))
        qi = e(nc.sbuf_tensor([P, D], f16))
        q0 = e(nc.sbuf_tensor([P, D], f16))
        q1 = e(nc.sbuf_tensor([P, D], f16))
        chunk0 = e(nc.sbuf_tensor([P, K * D], f16))
        chunk1f = e(nc.sbuf_tensor([P, NF1 * D], f16))
        chunk1b = e(nc.sbuf_tensor([P, (K - NF1) * D], u8))
        in0a_sem = e(nc.semaphore("in0a_sem"))   # gn0 left + logits p0-63
        in0b_sem = e(nc.semaphore("in0b_sem"))   # gn0 right half
        xfree_sem = e(nc.semaphore("xfree_sem")) # tile-0 add consumed lg
        in1_sem = e(nc.semaphore("in1_sem"))     # SWDGE accum gn1 done
        cmp_sem = e(nc.semaphore("cmp_sem"))
        dma_sem = e(nc.semaphore("dma_sem"))
        block = e(nc.Block(no_gpsimd_drain=True))

        H = D // 2
        NDMA = len(GROUPS0) + len(GROUPS1F) + len(GROUPS1B)

        @block.scalar
        def _(scalar: "bass.BassEngine"):
            # both logits halves on the scalar HWDGE ring: v5 showed the
            # SWDGE path (gpsimd preamble + first-byte + receipt) gated the
            # chain start at 13.3 us; a second HWDGE slot lands ~1.5 us
            # earlier
            scalar.dma_start(out=lg[0:N, :], in_=lg_d[:, :]).then_inc(in0a_sem, 16)
            scalar.dma_start(out=lg[N:P, :], in_=lg_d[:, :]).then_inc(in0a_sem, 16)
            for s, l in GROUPS1F:
                scalar.wait_ge(cmp_sem, _gate(1, s, l))
                scalar.dma_start(
                    out=out_a[P : 2 * P, D * s : D * (s + l)],
                    in_=chunk1f[:, D * s : D * (s + l)],
                ).then_inc(dma_sem, 16)
            for s, l in GROUPS1B[:-1]:
                scalar.wait_ge(cmp_sem, _gate(1, s, l))
                scalar.dma_start(
                    out=out_b[:, D * (s - NF1) : D * (s - NF1 + l)],
                    in_=chunk1b[:, D * (s - NF1) : D * (s - NF1 + l)],
                ).then_inc(dma_sem, 16)

        @block.gpsimd
        def _(gpsimd: "bass.BassEngine"):
            # tile-1 add for free: gn rows 128..255 accumulate onto lg
            gpsimd.wait_ge(xfree_sem, 1)
            gpsimd.dma_start(
                out=lg[:, :], in_=gn_d[P : 2 * P, :], accum_op=A.add
            ).then_inc(in1_sem, 16)

        @block.sync
        def _(sync: "bass.BassEngine"):
            sync.dma_start(out=gt0[:, 0:H], in_=gn_d[0:P, 0:H]).then_inc(
                in0a_sem, 16
            )
            sync.dma_start(out=gt0[:, H:D], in_=gn_d[0:P, H:D]).then_inc(
                in0b_sem, 16
            )
            for s, l in GROUPS0:
                sync.wait_ge(cmp_sem, _gate(0, s, l))
                sync.dma_start(
                    out=out_a[0:P, D * s : D * (s + l)],
                    in_=chunk0[:, D * s : D * (s + l)],
                ).then_inc(dma_sem, 16)
            s, l = GROUPS1B[-1]
            sync.wait_ge(cmp_sem, _gate(1, s, l))
            sync.dma_start(
                out=out_b[:, D * (s - NF1) : D * (s - NF1 + l)],
                in_=chunk1b[:, D * (s - NF1) : D * (s - NF1 + l)],
            ).then_inc(dma_sem, 16)
            sync.wait_ge(dma_sem, 16 * NDMA)

        @block.vector
        def _(vector: "bass.BassEngine"):
            def dr():
                vector.drain()

            xs = [x, lg]
            qs = [q0, q1]
            for i in range(NTILES):
                if i == 0:
                    vector.wait_ge(in0a_sem, 48)
                    vector.tensor_tensor(
                        x[:, 0:H], gt0[:, 0:H], lg[:, 0:H], op=A.add
                    )
                    vector.wait_ge(in0b_sem, 16)
                    vector.tensor_tensor(
                        x[:, H:D], gt0[:, H:D], lg[:, H:D], op=A.add
                    ).then_inc(xfree_sem, 1)
                    dr()
                else:
                    vector.wait_ge(in1_sem, 16)
                xi = xs[i][:]
                vector.max(v8[:], xi)
                dr()
                vector.match_replace(x2[:], v8[:], xi, -1e30)
                dr()
                vector.max(v16[:], x2[:])
                dr()
                vector.tensor_scalar(m[:], xi, v16[:, 7:8], None, op0=A.is_ge)
                dr()
                vector.tensor_tensor_scan(
                    qi[:], m[:], m[:], 0.0, op0=A.add, op1=A.bypass
                )
                dr()
                vector.tensor_tensor(qs[i][:], qi[:], m[:], op=A.mult)
                dr()
                for j in range(K):
                    if i == 0:
                        dst = chunk0[:, D * j : D * (j + 1)]
                    elif j < NF1:
                        dst = chunk1f[:, D * j : D * (j + 1)]
                    else:
                        dst = chunk1b[:, D * (j - NF1) : D * (j - NF1 + 1)]
                    vector.tensor_scalar(
                        dst, qs[i][:], float(j + 1), None, op0=A.is_equal
                    ).then_inc(cmp_sem, 1)
                vector.drain().then_inc(cmp_sem, 1)

    return nc


def _get_nc():
    if "nc" not in _CACHE:
        _CACHE["nc"] = _build_nc()
    return _CACHE["nc"]


def kernel(logits: np.ndarray, gn: np.ndarray) -> np.ndarray:
    from concourse.bass_utils import run_bass_kernel_spmd

    logits = np.ascontiguousarray(np.asarray(logits, dtype=np.float32))
    gn = np.asarray(gn, dtype=np.float32)
    assert logits.shape == (N, D) and gn.shape == (BS, N, D)

    nc = _get_nc()
    in_maps = []
    for c in range(NCORES):
        shard = np.ascontiguousarray(
            gn[c * BS_PER_CORE : (c + 1) * BS_PER_CORE].reshape(ROWS, D)
        )
        in_maps.append({"logits": logits, "gn": shard})

    res = run_bass_kernel_spmd(nc, in_maps, list(range(NCORES))).results
    outs = []
    for r in res:
        full = r["out_a"].astype(np.float32)          # [256, K*D]
        full[P:, NF1 * D :] = r["out_b"].astype(np.float32)
        outs.append(full.reshape(BS_PER_CORE, N, K, D))
    return np.concatenate(outs, axis=0)
